# revision 13
# baseline (speedup 1.0000x reference)
"""Trainium2 Bass kernel for nn_MaxCDFdp_multiclass.

Computes max over (class, probe) of |ECDF0 - ECDF1| where the ECDFs are
sigmoid-smoothed empirical CDFs of y_pred per class, for the two groups
defined by s in {0,1}.

v7: 5-probe windows + order-8 exponential-moment tails.  For
|z| >= 10*DELTA the sigmoid expansion sigma(z) = 1 - e^-z + e^-2z - ...
is accurate to ~e^(-(ORDER+1)*10*DELTA) per sample, and each tail term
FACTORIZES into  e^{-k t g_p} * sum_i e^{k t y_i}  -- per-tile/class
exponential moments the host computes in f64.  So the device evaluates
sigmoid on only W1=6 probes per (sample, class) (W2=22 for sparse
distribution-tail tiles), vs 56 in v3 and 100 naively.

Sharding: the per-class-sorted sample arrays are segmented globally
into tiles of <=128 rows whose per-class span fits the window, then
tiles are dealt round-robin to the 8 cores.

Device, per group of <=11 full tiles (DVE and GPSIMD share an SBUF port
so elementwise work runs ONLY on DVE -- concurrency halves both):
  DVE : diff[s,(t,c,w)] = A[s,t,c] + Dj[c,w]          (f32)
  ACT : sig = sigmoid(10*diff) -> bf16                (one op per group)
  PE  : per tile one matmul  ind8[128,8]^T @ sig -> [8, C*W]
        ind8 (shipped as bf16 packed in the f32 blob, bitcast on
        device) is the stationary operand; sig is the moving operand.
        Tile t lands on PSUM partition rows (2a, 2a+1) of column-group
        j = t%3 at free offset (t//12)*120: the matmul writes all 8
        rows but unused ind8 columns are zero and accumulate
        (start=False) onto regions pre-zeroed by zero-weight matmuls.
        Nothing is drained mid-kernel; wide tiles run right after the
        first full group so their PSUM bank drains early, off the
        critical tail.
  Drain: DVE copies the wide bank mid-kernel; ACT copies the full bank
        at the end; six per-(colgroup, bank) output DMAs spread across
        the sync HWDGE, scalar HWDGE, and gpsimd SWDGE rings, with the
        wide-bank DMAs fully hidden under compute.
Host: relocate each tile's [2, C, W] window into [2, C, P] at its
B offsets, add moment tails, sum over cores, divide by group counts,
abs, max.
"""

import os
from contextlib import ExitStack

import numpy as np

import concourse.bass as bass
import concourse.bacc as bacc
import concourse.tile as tile
from concourse import mybir
from concourse.bass_utils import run_bass_kernel_spmd

N, C, P = 50000, 20, 100
TEMP = 10.0
NCORES = 8
PART = 128
W1 = 5                 # probe window, full tiles
W2 = 22                # probe window, sparse (wide) tiles
DELTA = 0.065          # expansion validity margin in y units
ORDER = 8              # tail expansion order
CW1 = C * W1           # 120
CW2 = C * W2           # 440
BANK = 512             # f32 per PSUM bank per partition
NJ = 3                 # PE column-groups used (-> 3 output DMA rings)

_F32 = mybir.dt.float32
_BF16 = mybir.dt.bfloat16

_CACHED = {}


def _slot(t, wide):
    """tile index -> (colgroup j, partition pair a, bank, f32 offset)"""
    if wide:
        return t % NJ, (t // NJ) % 4, 1, BANK
    j, a, q = t % NJ, (t // NJ) % 4, t // (4 * NJ)
    if q < 4:
        return j, a, 0, q * CW1
    return j, a, 2, 2 * BANK + (q - 4) * CW1


def _group_sizes(T, first=6, rest=11):
    sizes = []
    if T:
        sizes.append(min(first, T))
        rem = T - sizes[0]
        while rem:
            g = min(rest, rem)
            sizes.append(g)
            rem -= g
    return sizes


def _grp_list(T):
    out = []
    i = 0
    for g in _group_sizes(T):
        out.append((i, g))
        i += g
    return out


def _build_bass(T1, T2):
    TT = T1 + T2
    dw1, dw2, iw = CW1, CW2, TT * 4  # ind8 packed as bf16 pairs in f32 cols
    ah = C // 2                      # A is shipped bf16: C bf16 = C//2 f32 cols
    g0n = _group_sizes(T1)[0]
    blob_w = dw1 + iw + g0n * ah + dw2 + T2 * ah + (T1 - g0n) * ah
    nc = bacc.Bacc(None, target_bir_lowering=False)
    b_d = nc.dram_tensor("b", [PART, blob_w], _F32, kind="ExternalInput")

    banks_used = {0}
    if T2:
        banks_used.add(1)
    for t in range(T1):
        banks_used.add(_slot(t, False)[2])
    nbank = max(banks_used) + 1
    o_d = nc.dram_tensor("o", [8 * NJ, nbank * BANK], _F32, kind="ExternalOutput")

    g1 = []
    i = 0
    for g in _group_sizes(T1):
        g1.append((i, g))
        i += g

    # last accumulating matmul per (j, bank) region gets stop=True;
    # wides run early (between full groups 0 and 1) in global order
    order_full = [t for t in range(T1)]
    last_in_region = {}
    for t in range(T2):
        j, a, b, off = _slot(t, True)
        last_in_region[(j, b)] = ("w", t)
    for t in order_full:
        j, a, b, off = _slot(t, False)
        last_in_region[(j, b)] = ("f", t)
    last_set = set(last_in_region.values())

    with ExitStack() as ctx:
        tc = ctx.enter_context(tile.TileContext(nc))
        constp = ctx.enter_context(tc.tile_pool(name="const", bufs=1))
        diffp = ctx.enter_context(tc.tile_pool(name="diff", bufs=3))
        sigp = ctx.enter_context(tc.tile_pool(name="sig", bufs=3))
        psump = ctx.enter_context(
            tc.tile_pool(name="psum", bufs=1, space=bass.MemorySpace.PSUM)
        )

        # zero stationary/moving for the region-clearing matmuls; also
        # feeds a dummy sigmoid that pulls the ACT table load forward
        zeros = constp.tile([PART, BANK], _BF16)
        nc.gpsimd.memset(zeros[:], 0.0)
        dummy_s = constp.tile([PART, 1], _F32)
        nc.scalar.activation(
            dummy_s[:],
            zeros[:, 0:1],
            mybir.ActivationFunctionType.Sigmoid,
            scale=TEMP,
        )

        accs = [psump.tile([PART, BANK], _F32, name=f"acc{b}") for b in range(nbank)]
        for j in range(NJ):
            for b in range(nbank):
                nc.tensor.matmul(
                    accs[b][32 * j : 32 * j + 8, :],
                    zeros[:, 0:8],
                    zeros[:, :],
                    start=True,
                    stop=False,
                    tile_position=(0, 32 * j),
                )

        blob = constp.tile([PART, blob_w], _F32)
        s1 = dw1 + iw + g0n * ah             # Dj1 + ind8 + A1 of group 0
        s2 = s1 + dw2 + T2 * ah              # + Dj2 + A2 (wide runs early)
        # chunked input: group 0's operands first, then per-group A slices
        # alternating between the two HWDGE rings so neither stream stalls
        # the group pipeline; the wide chunk goes third on sync
        nc.sync.dma_start(blob[:, 0:s1], b_d[:, 0:s1])
        a_marks = [s2]
        for k, (g0_, gn_) in enumerate(_grp_list(T1)[1:]):
            a0 = s2 + (g0_ - g0n) * ah
            a1_ = a0 + gn_ * ah
            a_marks.append(a1_)
            eng = nc.scalar if k % 2 == 1 else nc.sync
            if k == 2 and s1 < s2:
                nc.sync.dma_start(blob[:, s1:s2], b_d[:, s1:s2])
            eng.dma_start(blob[:, a0:a1_], b_d[:, a0:a1_])
        if len(a_marks) <= 3 and s1 < s2:
            nc.sync.dma_start(blob[:, s1:s2], b_d[:, s1:s2])
        dj1_sb = blob[:, 0:dw1].rearrange("p (c w) -> p c w", c=C)
        ind_r = (
            blob[:, dw1 : dw1 + iw]
            .bitcast(_BF16)
            .rearrange("p (t g) -> p t g", t=TT)
        )
        a1g0_sb = (
            blob[:, dw1 + iw : s1].bitcast(_BF16).rearrange("p (t c) -> p t c", t=g0n)
        )
        dj2_sb = blob[:, s1 : s1 + dw2].rearrange("p (c w) -> p c w", c=C)
        if T2:
            a2_sb = (
                blob[:, s1 + dw2 : s2]
                .bitcast(_BF16)
                .rearrange("p (t c) -> p t c", t=T2)
            )
        if T1 > g0n:
            a1r_sb = (
                blob[:, s2:].bitcast(_BF16).rearrange("p (t c) -> p t c", t=T1 - g0n)
            )

        def phase(groups, a_sb, a_base, dj_sb, W, wide, gcap, dtag, stag):
            CW = C * W
            for g0, gn in groups:
                diff = diffp.tile([PART, gcap, C, W], _F32, tag=dtag)
                a_v = (
                    a_sb[:, g0 - a_base : g0 - a_base + gn, :]
                    .unsqueeze(3)
                    .broadcast_to([PART, gn, C, W])
                )
                d_v = dj_sb[:].unsqueeze(1).broadcast_to([PART, gn, C, W])
                nc.vector.tensor_add(diff[:, 0:gn], a_v, d_v)

                sig = sigp.tile([PART, gcap, C, W], _BF16, tag=stag)
                nc.scalar.activation(
                    sig[:, 0:gn],
                    diff[:, 0:gn],
                    mybir.ActivationFunctionType.Sigmoid,
                    scale=TEMP,
                )
                sig_f = sig[:].rearrange("p t c w -> p t (c w)")
                for t in range(gn):
                    tloc = g0 + t
                    i = tloc + (T1 if wide else 0)
                    j, a, b, off = _slot(tloc, wide)
                    nc.tensor.matmul(
                        accs[b][32 * j : 32 * j + 8, off - b * BANK : off - b * BANK + CW],
                        ind_r[:, i, :],
                        sig_f[:, t, :],
                        start=False,
                        stop=(("w" if wide else "f", tloc) in last_set),
                        tile_position=(0, 32 * j),
                    )

        out_sb = constp.tile([PART, nbank * BANK], _F32)

        # groups 0-1 (A via chunks 1-2), then wides (chunk 3; their PSUM
        # bank drains early), then the remaining groups
        gcap = max(g for _, g in g1)
        rings = [nc.sync, nc.scalar, nc.gpsimd]
        phase(g1[:1], a1g0_sb, 0, dj1_sb, W1, False, g0n, "d1", "s1")
        for gi_, (g0_, gn_) in enumerate(g1[1:]):
            phase([(g0_, gn_)], a1r_sb, g0n, dj1_sb, W1, False, gcap, "d1b", "s1b")
            if T2 and gi_ == 0:
                phase([(0, T2)], a2_sb, 0, dj2_sb, W2, True, T2, "d2", "s2")
            if T2 and gi_ == 1:
                # wide-bank drain + DMAs issue mid-kernel, hidden under
                # compute (the wait is long satisfied by now)
                nc.vector.tensor_copy(out_sb[:, BANK : 2 * BANK], accs[1][:])
                for j in range(NJ):
                    rings[j].dma_start(
                        o_d[8 * j : 8 * j + 8, BANK : 2 * BANK],
                        out_sb[32 * j : 32 * j + 8, BANK : 2 * BANK],
                    )

        nc.scalar.activation(
            out_sb[:, 0:BANK], accs[0][:], mybir.ActivationFunctionType.Copy
        )
        if nbank > 2:
            nc.vector.tensor_copy(out_sb[:, 2 * BANK :], accs[2][:])
        for j in range(NJ):
            rings[j].dma_start(
                o_d[8 * j : 8 * j + 8, 0:BANK], out_sb[32 * j : 32 * j + 8, 0:BANK]
            )
            if nbank > 2:
                rings[j].dma_start(
                    o_d[8 * j : 8 * j + 8, 2 * BANK :],
                    out_sb[32 * j : 32 * j + 8, 2 * BANK :],
                )

    nc.finalize()
    return nc


def _get_nc(T1, T2):
    key = (T1, T2)
    if key not in _CACHED:
        _CACHED[key] = _build_bass(T1, T2)
    return _CACHED[key]


def _pack_bf16(x):
    """f32 array [..., 2k] -> bf16 pairs packed into f32 columns [..., k]"""
    import ml_dtypes

    b = x.astype(ml_dtypes.bfloat16).view(np.uint16)
    return b.view(np.uint32).view(np.float32)


# test.py reads this after calling kernel() for profiling info
LAST_RESULTS = None
LAST_DELTA = None


def kernel(y_pred: np.ndarray, s: np.ndarray) -> np.ndarray:
    global LAST_RESULTS, LAST_DELTA
    y = np.ascontiguousarray(np.asarray(y_pred), dtype=np.float32)
    s_np = np.asarray(s)
    assert y.shape == (N, C)

    mn = y.min(axis=0)
    mx = y.max(axis=0)
    step = (mx.astype(np.float64) - mn) / (P - 1)
    grid = mn.astype(np.float64)[:, None] + step[:, None] * np.arange(P)[None, :]

    srt0 = np.sort(y[s_np == 0], axis=0)
    srt1 = np.sort(y[s_np == 1], axis=0)
    n0, n1 = srt0.shape[0], srt1.shape[0]

    sm1 = (W1 - 2) * step - 2 * DELTA
    sm2 = (W2 - 2) * step - 2 * DELTA
    assert sm1.min() > 0.02 and sm2.min() > 0.02

    # global two-level segmentation, then deal tiles round-robin to cores
    fulls, wides = [], []
    for gi, (blk, n) in enumerate(((srt0, n0), (srt1, n1))):
        m = blk.shape[0]
        start = 0
        while start < m:
            end = min(start + PART, m)
            lim = m
            for c in range(C):
                e = np.searchsorted(blk[:, c], blk[start, c] + sm1[c], "right")
                lim = min(lim, e)
            if lim >= end:
                fulls.append((gi, blk[start:end]))
            else:
                lim = m
                for c in range(C):
                    e = np.searchsorted(blk[:, c], blk[start, c] + sm2[c], "right")
                    lim = min(lim, e)
                end = min(min(start + PART, m), max(lim, start + 1))
                wides.append((gi, blk[start:end]))
            start = end
    core_full = [fulls[r::NCORES] for r in range(NCORES)]
    core_wide = [wides[r::NCORES] for r in range(NCORES)]
    T1 = max(len(f) for f in core_full)
    T2 = max(len(w) for w in core_wide)
    assert T1 <= NJ * 4 * 5 and T2 <= NJ * 4, (T1, T2)
    TT = T1 + T2
    g0n = _group_sizes(T1)[0]

    dj1 = (step.astype(np.float32)[:, None] * np.arange(W1, dtype=np.float32)).astype(
        np.float32
    )
    dj2 = (step.astype(np.float32)[:, None] * np.arange(W2, dtype=np.float32)).astype(
        np.float32
    )
    dw1, dw2, iw = CW1, CW2, TT * 4
    ah = C // 2
    s1 = dw1 + iw + g0n * ah
    s2 = s1 + dw2 + T2 * ah
    blob_w = s2 + (T1 - g0n) * ah
    in_maps = []
    meta = []  # per core: list of (gi, B[C], cnt, up, lo, W, wide, t)
    for r in range(NCORES):
        A1 = np.zeros((PART, T1, C), np.float32)
        A2 = np.zeros((PART, max(T2, 1), C), np.float32)
        ind8 = np.zeros((PART, TT, 8), np.float32)
        tl = []
        for wide, (tiles, A, W) in enumerate(
            ((core_full[r], A1, W1), (core_wide[r], A2, W2))
        ):
            for t, (gi, vals) in enumerate(tiles):
                cnt = vals.shape[0]
                v64 = vals.astype(np.float64)
                ymax_t = v64.max(axis=0)
                B = np.ceil((ymax_t + DELTA - mn) / step).astype(np.int64) - W
                B = np.clip(B, 0, P - W)
                base = (mn + step * B).astype(np.float32)
                A[:cnt, t, :] = base[None, :] - vals
                A[cnt:, t, :] = base[None, :] - vals[-1]
                gslot = t + (T1 if wide else 0)
                j, a, b, off = _slot(t, bool(wide))
                ind8[:cnt, gslot, 2 * a + gi] = 1.0
                up = [np.exp(10 * k * v64).sum(axis=0) for k in range(1, ORDER + 1)]
                lo = [np.exp(-10 * k * v64).sum(axis=0) for k in range(1, ORDER + 1)]
                tl.append((gi, B, cnt, up, lo, W, bool(wide), t))
        meta.append(tl)
        blob = np.empty((PART, blob_w), np.float32)
        blob[:, 0:dw1] = np.broadcast_to(dj1.reshape(1, dw1), (PART, dw1))
        blob[:, dw1 : dw1 + iw] = _pack_bf16(ind8.reshape(PART, TT * 8))
        blob[:, dw1 + iw : s1] = _pack_bf16(A1[:, :g0n].reshape(PART, g0n * C))
        blob[:, s1 : s1 + dw2] = np.broadcast_to(dj2.reshape(1, dw2), (PART, dw2))
        blob[:, s1 + dw2 : s2] = _pack_bf16(A2[:, :T2].reshape(PART, T2 * C))
        blob[:, s2:] = _pack_bf16(A1[:, g0n:].reshape(PART, (T1 - g0n) * C))
        in_maps.append({"b": blob})

    nc = _get_nc(T1, T2)
    res = run_bass_kernel_spmd(
        nc,
        in_maps,
        core_ids=list(range(NCORES)),
        trace=bool(int(os.environ.get("BASS_KERNEL_TRACE", "0"))),
    )
    LAST_RESULTS = res

    # host assembly: windows + moment tails
    full = np.zeros((2, C, P), np.float64)
    egu, egl = [], []
    for c in range(C):
        g = grid[c]
        egu.append([np.exp(-10 * k * g) for k in range(1, ORDER + 1)])
        egl.append([np.exp(10 * k * g) for k in range(1, ORDER + 1)])
    for r in range(NCORES):
        o = res.results[r]["o"]  # [24, nbank*512] f32
        for gi, B, cnt, up, lo, W, wide, t in meta[r]:
            j, a, b, off = _slot(t, wide)
            win = (
                o[8 * j + 2 * a : 8 * j + 2 * a + 2, off : off + C * W]
                .astype(np.float64)
                .reshape(2, C, W)
            )
            for c in range(C):
                bb = int(B[c])
                full[:, c, bb : bb + W] += win[:, c]
                if bb + W < P:
                    add = np.float64(cnt)
                    for k in range(1, ORDER + 1):
                        add = add + (-1) ** k * egu[c][k - 1][bb + W :] * up[k - 1][c]
                    full[gi, c, bb + W :] += add
                if bb > 0:
                    add = np.zeros(bb, np.float64)
                    for k in range(1, ORDER + 1):
                        add = add - (-1) ** k * egl[c][k - 1][:bb] * lo[k - 1][c]
                    full[gi, c, :bb] += add
    delta = np.abs(full[0] / n0 - full[1] / n1)
    LAST_DELTA = delta
    return np.array(delta.max(), dtype=np.float32)


# revision 14
# speedup vs baseline: 1.0423x; 1.0423x over previous
"""Trainium2 Bass kernel for nn_MaxCDFdp_multiclass.

Computes max over (class, probe) of |ECDF0 - ECDF1| where the ECDFs are
sigmoid-smoothed empirical CDFs of y_pred per class, for the two groups
defined by s in {0,1}.

v7: 5-probe windows + order-8 exponential-moment tails.  For
|z| >= 10*DELTA the sigmoid expansion sigma(z) = 1 - e^-z + e^-2z - ...
is accurate to ~e^(-(ORDER+1)*10*DELTA) per sample, and each tail term
FACTORIZES into  e^{-k t g_p} * sum_i e^{k t y_i}  -- per-tile/class
exponential moments the host computes in f64.  So the device evaluates
sigmoid on only W1=6 probes per (sample, class) (W2=22 for sparse
distribution-tail tiles), vs 56 in v3 and 100 naively.

Sharding: the per-class-sorted sample arrays are segmented globally
into tiles of <=128 rows whose per-class span fits the window, then
tiles are dealt round-robin to the 8 cores.

Device, per group of <=11 full tiles (DVE and GPSIMD share an SBUF port
so elementwise work runs ONLY on DVE -- concurrency halves both):
  DVE : diff[s,(t,c,w)] = A[s,t,c] + Dj[c,w]          (f32)
  ACT : sig = sigmoid(10*diff) -> bf16                (one op per group)
  PE  : per tile one matmul  ind8[128,8]^T @ sig -> [8, C*W]
        ind8 (shipped as bf16 packed in the f32 blob, bitcast on
        device) is the stationary operand; sig is the moving operand.
        Tile t lands on PSUM partition rows (2a, 2a+1) of column-group
        j = t%3 at free offset (t//12)*120: the matmul writes all 8
        rows but unused ind8 columns are zero and accumulate
        (start=False) onto regions pre-zeroed by zero-weight matmuls.
        Nothing is drained mid-kernel; wide tiles run right after the
        first full group so their PSUM bank drains early, off the
        critical tail.
  Drain: DVE copies the wide bank mid-kernel; ACT copies the full bank
        at the end; six per-(colgroup, bank) output DMAs spread across
        the sync HWDGE, scalar HWDGE, and gpsimd SWDGE rings, with the
        wide-bank DMAs fully hidden under compute.
Host: relocate each tile's [2, C, W] window into [2, C, P] at its
B offsets, add moment tails, sum over cores, divide by group counts,
abs, max.
"""

import os
from contextlib import ExitStack

import numpy as np

import concourse.bass as bass
import concourse.bacc as bacc
import concourse.tile as tile
from concourse import mybir
from concourse.bass_utils import run_bass_kernel_spmd

N, C, P = 50000, 20, 100
TEMP = 10.0
NCORES = 8
PART = 128
W1 = 5                 # probe window, full tiles
W2 = 22                # probe window, sparse (wide) tiles
DELTA = 0.065          # expansion validity margin in y units
ORDER = 8              # tail expansion order
CW1 = C * W1           # 120
CW2 = C * W2           # 440
BANK = 512             # f32 per PSUM bank per partition
NJ = 3                 # PE column-groups used (-> 3 output DMA rings)

_F32 = mybir.dt.float32
_BF16 = mybir.dt.bfloat16

_CACHED = {}


def _slot(t, wide):
    """tile index -> (colgroup j, partition pair a, bank, f32 offset)"""
    if wide:
        return t % NJ, (t // NJ) % 4, 1, BANK
    j, a, q = t % NJ, (t // NJ) % 4, t // (4 * NJ)
    if q < 4:
        return j, a, 0, q * CW1
    return j, a, 2, 2 * BANK + (q - 4) * CW1


def _group_sizes(T, first=6, rest=11):
    sizes = []
    if T:
        sizes.append(min(first, T))
        rem = T - sizes[0]
        while rem:
            g = min(rest, rem)
            sizes.append(g)
            rem -= g
    return sizes


def _grp_list(T):
    out = []
    i = 0
    for g in _group_sizes(T):
        out.append((i, g))
        i += g
    return out


def _build_bass(T1, T2):
    TT = T1 + T2
    dw1, dw2, iw = CW1, CW2, TT * 4  # ind8 packed as bf16 pairs in f32 cols
    ah = C // 2                      # A is shipped bf16: C bf16 = C//2 f32 cols
    g0n = _group_sizes(T1)[0]
    blob_w = dw1 + iw + g0n * ah + dw2 + T2 * ah + (T1 - g0n) * ah
    nc = bacc.Bacc(None, target_bir_lowering=False)
    b_d = nc.dram_tensor("b", [PART, blob_w], _F32, kind="ExternalInput")

    banks_used = {0}
    if T2:
        banks_used.add(1)
    for t in range(T1):
        banks_used.add(_slot(t, False)[2])
    nbank = max(banks_used) + 1
    o_d = nc.dram_tensor("o", [8 * NJ, nbank * BANK], _F32, kind="ExternalOutput")

    g1 = []
    i = 0
    for g in _group_sizes(T1):
        g1.append((i, g))
        i += g

    # last accumulating matmul per (j, bank) region gets stop=True;
    # wides run early (between full groups 0 and 1) in global order
    order_full = [t for t in range(T1)]
    last_in_region = {}
    for t in range(T2):
        j, a, b, off = _slot(t, True)
        last_in_region[(j, b)] = ("w", t)
    for t in order_full:
        j, a, b, off = _slot(t, False)
        last_in_region[(j, b)] = ("f", t)
    last_set = set(last_in_region.values())

    with ExitStack() as ctx:
        tc = ctx.enter_context(tile.TileContext(nc))
        constp = ctx.enter_context(tc.tile_pool(name="const", bufs=1))
        diffp = ctx.enter_context(tc.tile_pool(name="diff", bufs=3))
        sigp = ctx.enter_context(tc.tile_pool(name="sig", bufs=3))
        psump = ctx.enter_context(
            tc.tile_pool(name="psum", bufs=1, space=bass.MemorySpace.PSUM)
        )

        # zero stationary/moving for the region-clearing matmuls; also
        # feeds a dummy sigmoid that pulls the ACT table load forward
        zeros = constp.tile([PART, BANK], _BF16)
        nc.gpsimd.memset(zeros[:], 0.0)
        dummy_s = constp.tile([PART, 1], _F32)
        nc.scalar.activation(
            dummy_s[:],
            zeros[:, 0:1],
            mybir.ActivationFunctionType.Sigmoid,
            scale=TEMP,
        )

        accs = [psump.tile([PART, BANK], _F32, name=f"acc{b}") for b in range(nbank)]
        for j in range(NJ):
            for b in range(nbank):
                nc.tensor.matmul(
                    accs[b][32 * j : 32 * j + 8, :],
                    zeros[:, 0:8],
                    zeros[:, :],
                    start=True,
                    stop=False,
                    tile_position=(0, 32 * j),
                )

        blob = constp.tile([PART, blob_w], _F32)
        s1 = dw1 + iw + g0n * ah             # Dj1 + ind8 + A1 of group 0
        s2 = s1 + dw2 + T2 * ah              # + Dj2 + A2 (wide runs early)
        # chunked input, all on one HWDGE ring in consumption order (a
        # concurrent stream on the other ring steals SDMA bandwidth from
        # chunk 1 and delays the whole pipeline): group 0's operands,
        # group 1's A, the wide chunk (phase order g0, g1, wide, g2, ...),
        # then the remaining per-group A slices
        nc.sync.dma_start(blob[:, 0:s1], b_d[:, 0:s1])
        for k, (g0_, gn_) in enumerate(_grp_list(T1)[1:]):
            a0 = s2 + (g0_ - g0n) * ah
            a1_ = a0 + gn_ * ah
            if k == 1 and s1 < s2:
                nc.sync.dma_start(blob[:, s1:s2], b_d[:, s1:s2])
            nc.sync.dma_start(blob[:, a0:a1_], b_d[:, a0:a1_])
        if len(_grp_list(T1)) <= 2 and s1 < s2:
            nc.sync.dma_start(blob[:, s1:s2], b_d[:, s1:s2])
        dj1_sb = blob[:, 0:dw1].rearrange("p (c w) -> p c w", c=C)
        ind_r = (
            blob[:, dw1 : dw1 + iw]
            .bitcast(_BF16)
            .rearrange("p (t g) -> p t g", t=TT)
        )
        a1g0_sb = (
            blob[:, dw1 + iw : s1].bitcast(_BF16).rearrange("p (t c) -> p t c", t=g0n)
        )
        dj2_sb = blob[:, s1 : s1 + dw2].rearrange("p (c w) -> p c w", c=C)
        if T2:
            a2_sb = (
                blob[:, s1 + dw2 : s2]
                .bitcast(_BF16)
                .rearrange("p (t c) -> p t c", t=T2)
            )
        if T1 > g0n:
            a1r_sb = (
                blob[:, s2:].bitcast(_BF16).rearrange("p (t c) -> p t c", t=T1 - g0n)
            )

        def phase(groups, a_sb, a_base, dj_sb, W, wide, gcap, dtag, stag):
            CW = C * W
            for g0, gn in groups:
                diff = diffp.tile([PART, gcap, C, W], _F32, tag=dtag)
                a_v = (
                    a_sb[:, g0 - a_base : g0 - a_base + gn, :]
                    .unsqueeze(3)
                    .broadcast_to([PART, gn, C, W])
                )
                d_v = dj_sb[:].unsqueeze(1).broadcast_to([PART, gn, C, W])
                nc.vector.tensor_add(diff[:, 0:gn], a_v, d_v)

                sig = sigp.tile([PART, gcap, C, W], _BF16, tag=stag)
                nc.scalar.activation(
                    sig[:, 0:gn],
                    diff[:, 0:gn],
                    mybir.ActivationFunctionType.Sigmoid,
                    scale=TEMP,
                )
                sig_f = sig[:].rearrange("p t c w -> p t (c w)")
                for t in range(gn):
                    tloc = g0 + t
                    i = tloc + (T1 if wide else 0)
                    j, a, b, off = _slot(tloc, wide)
                    nc.tensor.matmul(
                        accs[b][32 * j : 32 * j + 8, off - b * BANK : off - b * BANK + CW],
                        ind_r[:, i, :],
                        sig_f[:, t, :],
                        start=False,
                        stop=(("w" if wide else "f", tloc) in last_set),
                        tile_position=(0, 32 * j),
                    )

        out_sb = constp.tile([PART, nbank * BANK], _F32)

        # groups 0-1 (A via chunks 1-2), then wides (chunk 3; their PSUM
        # bank drains early), then the remaining groups
        gcap = max(g for _, g in g1)
        rings = [nc.sync, nc.scalar, nc.gpsimd]
        phase(g1[:1], a1g0_sb, 0, dj1_sb, W1, False, g0n, "d1", "s1")
        for gi_, (g0_, gn_) in enumerate(g1[1:]):
            phase([(g0_, gn_)], a1r_sb, g0n, dj1_sb, W1, False, gcap, "d1b", "s1b")
            if T2 and gi_ == 0:
                phase([(0, T2)], a2_sb, 0, dj2_sb, W2, True, T2, "d2", "s2")
            if T2 and gi_ == 1:
                # wide-bank drain + DMAs issue mid-kernel, hidden under
                # compute (the wait is long satisfied by now)
                nc.vector.tensor_copy(out_sb[:, BANK : 2 * BANK], accs[1][:])
                for j in range(NJ):
                    rings[j].dma_start(
                        o_d[8 * j : 8 * j + 8, BANK : 2 * BANK],
                        out_sb[32 * j : 32 * j + 8, BANK : 2 * BANK],
                    )

        nc.scalar.activation(
            out_sb[:, 0:BANK], accs[0][:], mybir.ActivationFunctionType.Copy
        )
        if nbank > 2:
            nc.vector.tensor_copy(out_sb[:, 2 * BANK :], accs[2][:])
        for j in range(NJ):
            rings[j].dma_start(
                o_d[8 * j : 8 * j + 8, 0:BANK], out_sb[32 * j : 32 * j + 8, 0:BANK]
            )
            if nbank > 2:
                rings[j].dma_start(
                    o_d[8 * j : 8 * j + 8, 2 * BANK :],
                    out_sb[32 * j : 32 * j + 8, 2 * BANK :],
                )

    nc.finalize()
    return nc


def _get_nc(T1, T2):
    key = (T1, T2)
    if key not in _CACHED:
        _CACHED[key] = _build_bass(T1, T2)
    return _CACHED[key]


def _pack_bf16(x):
    """f32 array [..., 2k] -> bf16 pairs packed into f32 columns [..., k]"""
    import ml_dtypes

    b = x.astype(ml_dtypes.bfloat16).view(np.uint16)
    return b.view(np.uint32).view(np.float32)


# test.py reads this after calling kernel() for profiling info
LAST_RESULTS = None
LAST_DELTA = None


def kernel(y_pred: np.ndarray, s: np.ndarray) -> np.ndarray:
    global LAST_RESULTS, LAST_DELTA
    y = np.ascontiguousarray(np.asarray(y_pred), dtype=np.float32)
    s_np = np.asarray(s)
    assert y.shape == (N, C)

    mn = y.min(axis=0)
    mx = y.max(axis=0)
    step = (mx.astype(np.float64) - mn) / (P - 1)
    grid = mn.astype(np.float64)[:, None] + step[:, None] * np.arange(P)[None, :]

    srt0 = np.sort(y[s_np == 0], axis=0)
    srt1 = np.sort(y[s_np == 1], axis=0)
    n0, n1 = srt0.shape[0], srt1.shape[0]

    sm1 = (W1 - 2) * step - 2 * DELTA
    sm2 = (W2 - 2) * step - 2 * DELTA
    assert sm1.min() > 0.02 and sm2.min() > 0.02

    # global two-level segmentation, then deal tiles round-robin to cores
    fulls, wides = [], []
    for gi, (blk, n) in enumerate(((srt0, n0), (srt1, n1))):
        m = blk.shape[0]
        start = 0
        while start < m:
            end = min(start + PART, m)
            lim = m
            for c in range(C):
                e = np.searchsorted(blk[:, c], blk[start, c] + sm1[c], "right")
                lim = min(lim, e)
            if lim >= end:
                fulls.append((gi, blk[start:end]))
            else:
                lim = m
                for c in range(C):
                    e = np.searchsorted(blk[:, c], blk[start, c] + sm2[c], "right")
                    lim = min(lim, e)
                end = min(min(start + PART, m), max(lim, start + 1))
                wides.append((gi, blk[start:end]))
            start = end
    core_full = [fulls[r::NCORES] for r in range(NCORES)]
    core_wide = [wides[r::NCORES] for r in range(NCORES)]
    T1 = max(len(f) for f in core_full)
    T2 = max(len(w) for w in core_wide)
    assert T1 <= NJ * 4 * 5 and T2 <= NJ * 4, (T1, T2)
    TT = T1 + T2
    g0n = _group_sizes(T1)[0]

    dj1 = (step.astype(np.float32)[:, None] * np.arange(W1, dtype=np.float32)).astype(
        np.float32
    )
    dj2 = (step.astype(np.float32)[:, None] * np.arange(W2, dtype=np.float32)).astype(
        np.float32
    )
    dw1, dw2, iw = CW1, CW2, TT * 4
    ah = C // 2
    s1 = dw1 + iw + g0n * ah
    s2 = s1 + dw2 + T2 * ah
    blob_w = s2 + (T1 - g0n) * ah
    in_maps = []
    meta = []  # per core: list of (gi, B[C], cnt, up, lo, W, wide, t)
    for r in range(NCORES):
        A1 = np.zeros((PART, T1, C), np.float32)
        A2 = np.zeros((PART, max(T2, 1), C), np.float32)
        ind8 = np.zeros((PART, TT, 8), np.float32)
        tl = []
        for wide, (tiles, A, W) in enumerate(
            ((core_full[r], A1, W1), (core_wide[r], A2, W2))
        ):
            for t, (gi, vals) in enumerate(tiles):
                cnt = vals.shape[0]
                v64 = vals.astype(np.float64)
                ymax_t = v64.max(axis=0)
                B = np.ceil((ymax_t + DELTA - mn) / step).astype(np.int64) - W
                B = np.clip(B, 0, P - W)
                base = (mn + step * B).astype(np.float32)
                A[:cnt, t, :] = base[None, :] - vals
                A[cnt:, t, :] = base[None, :] - vals[-1]
                gslot = t + (T1 if wide else 0)
                j, a, b, off = _slot(t, bool(wide))
                ind8[:cnt, gslot, 2 * a + gi] = 1.0
                up = [np.exp(10 * k * v64).sum(axis=0) for k in range(1, ORDER + 1)]
                lo = [np.exp(-10 * k * v64).sum(axis=0) for k in range(1, ORDER + 1)]
                tl.append((gi, B, cnt, up, lo, W, bool(wide), t))
        meta.append(tl)
        blob = np.empty((PART, blob_w), np.float32)
        blob[:, 0:dw1] = np.broadcast_to(dj1.reshape(1, dw1), (PART, dw1))
        blob[:, dw1 : dw1 + iw] = _pack_bf16(ind8.reshape(PART, TT * 8))
        blob[:, dw1 + iw : s1] = _pack_bf16(A1[:, :g0n].reshape(PART, g0n * C))
        blob[:, s1 : s1 + dw2] = np.broadcast_to(dj2.reshape(1, dw2), (PART, dw2))
        blob[:, s1 + dw2 : s2] = _pack_bf16(A2[:, :T2].reshape(PART, T2 * C))
        blob[:, s2:] = _pack_bf16(A1[:, g0n:].reshape(PART, (T1 - g0n) * C))
        in_maps.append({"b": blob})

    nc = _get_nc(T1, T2)
    res = run_bass_kernel_spmd(
        nc,
        in_maps,
        core_ids=list(range(NCORES)),
        trace=bool(int(os.environ.get("BASS_KERNEL_TRACE", "0"))),
    )
    LAST_RESULTS = res

    # host assembly: windows + moment tails
    full = np.zeros((2, C, P), np.float64)
    egu, egl = [], []
    for c in range(C):
        g = grid[c]
        egu.append([np.exp(-10 * k * g) for k in range(1, ORDER + 1)])
        egl.append([np.exp(10 * k * g) for k in range(1, ORDER + 1)])
    for r in range(NCORES):
        o = res.results[r]["o"]  # [24, nbank*512] f32
        for gi, B, cnt, up, lo, W, wide, t in meta[r]:
            j, a, b, off = _slot(t, wide)
            win = (
                o[8 * j + 2 * a : 8 * j + 2 * a + 2, off : off + C * W]
                .astype(np.float64)
                .reshape(2, C, W)
            )
            for c in range(C):
                bb = int(B[c])
                full[:, c, bb : bb + W] += win[:, c]
                if bb + W < P:
                    add = np.float64(cnt)
                    for k in range(1, ORDER + 1):
                        add = add + (-1) ** k * egu[c][k - 1][bb + W :] * up[k - 1][c]
                    full[gi, c, bb + W :] += add
                if bb > 0:
                    add = np.zeros(bb, np.float64)
                    for k in range(1, ORDER + 1):
                        add = add - (-1) ** k * egl[c][k - 1][:bb] * lo[k - 1][c]
                    full[gi, c, :bb] += add
    delta = np.abs(full[0] / n0 - full[1] / n1)
    LAST_DELTA = delta
    return np.array(delta.max(), dtype=np.float32)
